# revision 15
# baseline (speedup 1.0000x reference)
"""Masked community-RNN kernel for one TRN2 chip (8 NeuronCores).

Model (T=100 steps, B=128 batch, H=2048 hidden, 4 modules):
    h_{t+1} = tanh(x_t @ Wih.T + b_ih + h_t @ Whh.T + b_hh)
    out_t   = h_{t+1} @ Wout.T + b_out          (then split into 4 modules)

Parallelization: hidden-dimension sharding.  Core c owns 256 hidden rows
(2 chunks of 128).  Each step every core computes its slice of the new h
(full batch), then broadcasts the bf16 slice [128p x 256] directly into the
SBUF of the 7 peers via remote_dma_broadcast (one single-destination
"broadcast" per physical tpb-delta d=1..7, each on its own SDMA engine
pair).  No ncfw collectives on the critical path.

Slot layout (XOR-rotated so the SPMD program is fully static): h buffer
holds 8 slot-pairs of 256 cols; slot-pair s on core c contains the h-slice
of logical core (c XOR LAM[s]) where LAM = [0,1,2,3,6,7,4,5] is the
logical-XOR delta implemented by physical tpb-delta s (TRN2 NC base map;
chassis-dependent XOR constants cancel in relative deltas — verified by an
on-device probe).  Slot 0 is the core's own slice: tanh writes it in place
and the 7 sends read it; no self-loopback DMA.

Weights are fed per-core with K-blocks pre-permuted to match the rotated
slot order, so every matmul / DMA / activation uses static addressing and
all 8 cores run one identical program with different input data.

Per step per core: 40 z-matmuls (4x2 input-proj + 16x2 recurrent, N=128,
bf16, f32 PSUM accum) arrival-ordered by slot with per-round semaphore
waits, 2 tanh activations (PSUM->SBUF bf16, bias fused), 2 readout matmuls
(partial over own 256 hidden cols), 7 remote broadcasts, one 128KB x-tile
prefetch and one 32KB partial-readout store.  Host sums the 8 readout
partials (each core adds b_out/8 so the sum carries one b_out).
"""

import numpy as np
import ml_dtypes

import concourse.bass as bass
import concourse.bacc as bacc
import concourse.mybir as mybir
from concourse.bass_utils import run_bass_kernel_spmd

BF16 = ml_dtypes.bfloat16

NCORES = 8
T_FULL = 100
B = 128
H = 2048          # total hidden
HSLICE = 256      # hidden rows per core
IN_TOT = 512
OUT_TOT = 64
NKH = 16          # K-chunks over hidden
NKX = 4           # K-chunks over input
XRING = 4         # x prefetch depth (steps)

# physical tpb-delta d -> logical core XOR delta (TRN2 NC base map)
LAM = [0, 1, 2, 3, 6, 7, 4, 5]


def build(T=T_FULL, t_io=None):
    t_io = t_io or T
    nc = bacc.Bacc(
        num_devices=NCORES,
        target_bir_lowering=False,
        dynamic_dma_scratch_size=65536,
    )
    f32, bf16 = mybir.dt.float32, mybir.dt.bfloat16

    # per-core inputs (same shapes everywhere, different data)
    whh = nc.dram_tensor("whh", [128, 2 * NKH * 128], bf16, kind="ExternalInput")
    wih = nc.dram_tensor("wih", [128, 2 * NKX * 128], bf16, kind="ExternalInput")
    wout = nc.dram_tensor("wout", [128, 2 * OUT_TOT], bf16, kind="ExternalInput")
    bias = nc.dram_tensor("bias", [128, 2], f32, kind="ExternalInput")
    bout = nc.dram_tensor("bout", [OUT_TOT, 1], f32, kind="ExternalInput")
    xt = nc.dram_tensor("xt", [t_io, IN_TOT, B], bf16, kind="ExternalInput")
    out_p = nc.dram_tensor("out_p", [t_io, OUT_TOT, B], f32, kind="ExternalOutput")

    # semaphore thresholds (all static)
    def act_after(t):          # act_sem value once tanh of step t retired
        return 2 + 2 * (t + 1)  # +2 from the initial h memset

    def recv_thresh(t):        # recv[d][t&1] needed before step-t matmuls
        return 2 * ((t + 1) // 2)

    import contextlib
    with contextlib.ExitStack() as stk:
        E = stk.enter_context
        w_sem = E(nc.semaphore("w_sem"))
        x_sem = [E(nc.semaphore(f"x_sem_{s}")) for s in range(XRING)]
        out_sem = [E(nc.semaphore(f"out_sem_{par}")) for par in range(2)]
        prep_sem = E(nc.semaphore("prep_sem"))
        send_sem = [E(nc.semaphore(f"send_sem_{par}")) for par in range(2)]
        act_sem = E(nc.semaphore("act_sem"))
        pe_z_sem = E(nc.semaphore("pe_z_sem"))
        pe_r_sem = E(nc.semaphore("pe_r_sem"))
        dve_sem = E(nc.semaphore("dve_sem"))
        wh_sb = E(nc.sbuf_tensor("wh_sb", [128, 2 * NKH * 128], bf16))
        wih_sb = E(nc.sbuf_tensor("wih_sb", [128, 2 * NKX * 128], bf16))
        wout_sb = E(nc.sbuf_tensor("wout_sb", [128, 2 * OUT_TOT], bf16))
        bias_sb = E(nc.sbuf_tensor("bias_sb", [128, 2], f32))
        bout_sb = E(nc.sbuf_tensor("bout_sb", [OUT_TOT, 1], f32))
        hall0 = E(nc.sbuf_tensor("hall0", [128, H], bf16))
        hall1 = E(nc.sbuf_tensor("hall1", [128, H], bf16))
        x_sb = E(nc.sbuf_tensor("x_sb", [128, XRING * IN_TOT], bf16))
        stage = E(nc.sbuf_tensor("stage", [OUT_TOT, 2 * B], f32))
        pz00 = E(nc.psum_tensor("pz00", [128, B], f32))
        pz10 = E(nc.psum_tensor("pz10", [128, B], f32))
        pz01 = E(nc.psum_tensor("pz01", [128, B], f32))
        pz11 = E(nc.psum_tensor("pz11", [128, B], f32))
        pr0 = E(nc.psum_tensor("pr0", [OUT_TOT, B], f32))
        pr1 = E(nc.psum_tensor("pr1", [OUT_TOT, B], f32))
        recv = {d: [E(nc.semaphore(f"recv_{d}_{par}")) for par in range(2)]
                for d in range(1, 8)}

        hall = [hall0, hall1]
        pz = [[pz00, pz01], [pz10, pz11]]   # pz[m][parity]
        pr = [pr0, pr1]

        def wh_tile(j, m):     # lhsT for K-slot j (0..15), M-chunk m
            c0 = (2 * j + m) * 128
            return wh_sb[:, c0:c0 + 128]

        def wih_tile(k, m):
            c0 = (2 * k + m) * 128
            return wih_sb[:, c0:c0 + 128]

        def wout_tile(m):
            return wout_sb[:, m * OUT_TOT:(m + 1) * OUT_TOT]

        with nc.Block() as block:

            @block.gpsimd
            def _(gp):
                gp.dma_start(out=wh_sb[:, :], in_=whh[:, :]).then_inc(w_sem, 16)
                gp.dma_start(out=wih_sb[:, :], in_=wih[:, :]).then_inc(w_sem, 16)
                gp.dma_start(out=wout_sb[:, :], in_=wout[:, :]).then_inc(w_sem, 16)
                gp.dma_start(out=bias_sb[:, :], in_=bias[:, :]).then_inc(w_sem, 16)
                gp.dma_start(out=bout_sb[:, :], in_=bout[:, :]).then_inc(w_sem, 16)
                gp.memset(hall[0][:, :], 0).then_inc(act_sem, 2)
                for t in range(T):
                    dst = hall[(t + 1) & 1]
                    for d in range(1, 8):
                        rdests = [None] * 8
                        rdests[d] = (0, d)
                        gp.remote_dma_broadcast(
                            out_ap=dst[:, 256 * d:256 * d + 256],
                            in_ap=dst[:, 0:256],
                            remote_sem=recv[d][(t + 1) & 1],
                            local_sem=send_sem[(t + 1) & 1],
                            rdests=rdests,
                        ).then_inc(prep_sem, 1)
                    gp.wait_ge(prep_sem, 7 * (t + 1))
                    gp.wait_ge(act_sem, act_after(t))
                    gp.trigger_dma(count=7)

            @block.sync
            def _(sy):
                def x_dma(tp):
                    src = xt[tp % t_io].rearrange("(k p) b -> p k b", p=128)
                    slot = tp % XRING
                    dst = x_sb[:, slot * IN_TOT:(slot + 1) * IN_TOT]
                    sy.dma_start(
                        out=dst.rearrange("p (k b) -> p k b", k=NKX),
                        in_=src,
                    ).then_inc(x_sem[slot], 16)

                for tp in range(min(XRING, T)):
                    x_dma(tp)
                for t in range(T):
                    tp = t + XRING
                    if tp < T:
                        sy.wait_ge(pe_z_sem, t + 1)
                        x_dma(tp)
                    sy.wait_ge(dve_sem, t + 1)
                    sy.dma_start(
                        out=out_p[t % t_io],
                        in_=stage[:, (t & 1) * B:(t & 1) * B + B],
                    ).then_inc(out_sem[t & 1], 16)

            @block.tensor
            def _(pe):
                pe.wait_ge(w_sem, 80)
                for t in range(T):
                    p = t & 1
                    slot = t % XRING
                    hbuf = hall[p]
                    # --- input projection (opens the accumulation groups)
                    pe.wait_ge(x_sem[slot], 16 * (t // XRING + 1))
                    if t >= 2:
                        pe.wait_ge(act_sem, act_after(t - 2))
                    for m in range(2):
                        for k in range(NKX):
                            pe.matmul(
                                pz[m][p][:, :],
                                wih_tile(k, m),
                                x_sb[:, slot * IN_TOT + 128 * k:
                                     slot * IN_TOT + 128 * (k + 1)],
                                start=(k == 0), stop=False,
                            )
                    # --- own slice (slot-pair 0)
                    pe.wait_ge(act_sem, act_after(t - 1))
                    for q in range(2):
                        for m in range(2):
                            pe.matmul(
                                pz[m][p][:, :], wh_tile(q, m),
                                hbuf[:, 128 * q:128 * (q + 1)],
                                start=False, stop=False,
                            )
                    # --- readout of h_t (tanh output of step t-1, slot 0)
                    if t >= 1:
                        if t >= 3:
                            pe.wait_ge(dve_sem, t - 2)
                        rp = (t - 1) & 1
                        pe.matmul(pr[rp][:, :], wout_tile(0),
                                  hbuf[:, 0:128], start=True, stop=False)
                        pe.matmul(pr[rp][:, :], wout_tile(1),
                                  hbuf[:, 128:256], start=False, stop=True,
                                  ).then_inc(pe_r_sem, 1)
                    # --- remote slices, in arrival-friendly order
                    for s in range(1, 8):
                        th = recv_thresh(t)
                        if th > 0:
                            pe.wait_ge(recv[s][p], th)
                        for q in range(2):
                            for m in range(2):
                                last = (s == 7 and q == 1)
                                ins = pe.matmul(
                                    pz[m][p][:, :], wh_tile(2 * s + q, m),
                                    hbuf[:, 256 * s + 128 * q:
                                         256 * s + 128 * (q + 1)],
                                    start=False, stop=last,
                                )
                                if last and m == 1:
                                    ins.then_inc(pe_z_sem, 1)
                    # end step
                # final readout (h_T in hall[T&1] slot 0)
                pe.wait_ge(act_sem, act_after(T - 1))
                pe.wait_ge(dve_sem, T - 2)
                rp = (T - 1) & 1
                hbuf = hall[T & 1]
                pe.matmul(pr[rp][:, :], wout_tile(0), hbuf[:, 0:128],
                          start=True, stop=False)
                pe.matmul(pr[rp][:, :], wout_tile(1), hbuf[:, 128:256],
                          start=False, stop=True).then_inc(pe_r_sem, 1)

            @block.scalar
            def _(act):
                for t in range(T):
                    p = t & 1
                    act.wait_ge(pe_z_sem, t + 1)
                    if t >= 2:
                        act.wait_ge(send_sem[(t + 1) & 1], 112 * (t // 2))
                    for m in range(2):
                        act.activation(
                            out=hall[1 - p][:, 128 * m:128 * (m + 1)],
                            in_=pz[m][p][:, :],
                            func=mybir.ActivationFunctionType.Tanh,
                            bias=bias_sb[:, m:m + 1],
                        ).then_inc(act_sem, 1)

            @block.vector
            def _(ve):
                for t in range(T):
                    ve.wait_ge(pe_r_sem, t + 1)
                    if t >= 2:
                        ve.wait_ge(out_sem[t & 1], 16 * (t // 2))
                    ve.tensor_scalar_add(
                        out=stage[:, (t & 1) * B:(t & 1) * B + B],
                        in0=pr[t & 1][:, :],
                        scalar1=bout_sb[:, 0:1],
                    ).then_inc(dve_sem, 1)

    return nc


def prep_in_maps(x, w_ih, b_ih, w_hh, b_hh, w_out, b_out,
                 input_mask, hh_mask, out_mask, T=T_FULL):
    """Host-side shard/pack: fold masks, cast bf16, pre-transpose, rotate."""
    f32 = np.float32
    Wih = (np.asarray(w_ih, f32) * np.asarray(input_mask, f32))
    Whh = (np.asarray(w_hh, f32) * np.asarray(hh_mask, f32))
    Wout = (np.asarray(w_out, f32) * np.asarray(out_mask, f32))
    bsum = np.asarray(b_ih, f32) + np.asarray(b_hh, f32)
    bo = np.asarray(b_out, f32)

    x = np.asarray(x, f32)[:T]
    # [T, IN, B] bf16, contiguous
    xT = np.ascontiguousarray(x.transpose(0, 2, 1)).astype(BF16)

    in_maps = []
    for c in range(NCORES):
        r0 = HSLICE * c
        # whh tiles: K-slot j -> global hidden in-chunk g
        wh = np.empty((128, 2 * NKH, 128), BF16)
        for j in range(NKH):
            sp, q = j // 2, j % 2
            u = c ^ LAM[sp]
            g = 2 * u + q
            for m in range(2):
                blk = Whh[r0 + 128 * m: r0 + 128 * (m + 1),
                          128 * g: 128 * (g + 1)]
                wh[:, 2 * j + m, :] = blk.T.astype(BF16)
        wihp = np.empty((128, 2 * NKX, 128), BF16)
        for k in range(NKX):
            for m in range(2):
                blk = Wih[r0 + 128 * m: r0 + 128 * (m + 1),
                          128 * k: 128 * (k + 1)]
                wihp[:, 2 * k + m, :] = blk.T.astype(BF16)
        wo = np.empty((128, 2, OUT_TOT), BF16)
        for m in range(2):
            wo[:, m, :] = Wout[:, r0 + 128 * m: r0 + 128 * (m + 1)].T.astype(BF16)
        bias = np.stack([bsum[r0:r0 + 128], bsum[r0 + 128:r0 + 256]],
                        axis=1).astype(f32)
        in_maps.append({
            "whh": np.ascontiguousarray(wh.reshape(128, 2 * NKH * 128)),
            "wih": np.ascontiguousarray(wihp.reshape(128, 2 * NKX * 128)),
            "wout": np.ascontiguousarray(wo.reshape(128, 2 * OUT_TOT)),
            "bias": np.ascontiguousarray(bias),
            "bout": np.ascontiguousarray((bo / NCORES).reshape(OUT_TOT, 1)),
            "xt": xT,
        })
    return in_maps


def assemble(results, T=T_FULL):
    """Sum per-core partial readouts -> [T, 4, B, 16] f32."""
    acc = np.zeros((T, OUT_TOT, B), np.float32)
    for r in results:
        acc += r["out_p"]
    # out[t, m, b, o] = acc[t, m*16+o, b]
    return np.ascontiguousarray(
        acc.reshape(T, 4, 16, B).transpose(0, 1, 3, 2))


_CACHE = {}
_LAST_RESULT = None


def kernel(**inputs) -> np.ndarray:
    global _LAST_RESULT
    import os
    T = inputs["x"].shape[0]
    if T not in _CACHE:
        nc = build(T)
        nc.finalize()
        _CACHE[T] = nc
    nc = _CACHE[T]
    in_maps = prep_in_maps(T=T, **inputs)
    trace = bool(int(os.environ.get("BASS_RNN_TRACE", "0")))
    res = run_bass_kernel_spmd(
        nc, in_maps, core_ids=list(range(NCORES)), trace=trace
    )
    _LAST_RESULT = res
    return assemble(res.results, T=T)


# revision 19
# speedup vs baseline: 1.4896x; 1.4896x over previous
"""Masked community-RNN kernel for one TRN2 chip (8 NeuronCores).

Model (T=100 steps, B=128 batch, H=2048 hidden, 4 modules):
    h_{t+1} = tanh(x_t @ Wih.T + b_ih + h_t @ Whh.T + b_hh)
    out_t   = h_{t+1} @ Wout.T + b_out          (then split into 4 modules)

Parallelization: hidden-dimension sharding.  Core c owns 256 hidden rows
(2 chunks of 128).  Each step every core computes its slice of the new h
for the full batch, applies tanh (PSUM f32 -> SBUF bf16, bias fused), and
ships the bf16 slice [128p x 256] straight into the SBUF of all 8 cores
(self included, via loopback) with a single remote_dma_broadcast
instruction; receivers take one semaphore wait per step.  No ncfw
collectives anywhere on the critical path.

The gathered h buffer uses a global slot layout keyed by the sender's
logical id: each sender places its slice at column 256*id on every
receiver (one dynamic access pattern, offset register loaded once from a
per-core id input).  All other addressing is static, so the 8 cores run
one identical SPMD program with different input data.

Per step per core: 40 z-matmuls (4x2 input-proj + 16x2 recurrent, N=128,
bf16 operands, f32 PSUM accumulation, parity-alternating PSUM banks),
2 tanh activations, 2 readout matmuls (partial over the core's own 256
hidden columns; host sums the 8 partials, each core adds b_out/8), one
broadcast + trigger, one 128KB x-tile prefetch and one 32KB partial store.

NOTE on this environment: execution cost here is dominated by a fixed
~20-40us per *instruction* (measured: 40 back-to-back N=128 matmuls with
no waits run at ~1.8ms/step), so the design minimizes instruction count
(one broadcast instead of 7, one arrival wait instead of 7, minimal
semaphore waits) rather than classic roofline overlap.
"""

import numpy as np
import ml_dtypes

import concourse.bass as bass
import concourse.bacc as bacc
import concourse.mybir as mybir
from concourse.bass_utils import run_bass_kernel_spmd

BF16 = ml_dtypes.bfloat16

NCORES = 8
T_FULL = 100
B = 128
H = 2048          # total hidden
HSLICE = 256      # hidden rows per core
IN_TOT = 512
OUT_TOT = 64
NKH = 16          # K-chunks over hidden
NKX = 4           # K-chunks over input
XRING = 4         # x prefetch depth (steps)

# physical tpb-delta d -> logical core XOR delta (TRN2 NC base map)
LAM = [0, 1, 2, 3, 6, 7, 4, 5]


VARIANT = "full"


def build(T=T_FULL, t_io=None):
    t_io = t_io or T
    variant = VARIANT
    nc = bacc.Bacc(
        num_devices=NCORES,
        target_bir_lowering=False,
        dynamic_dma_scratch_size=65536,
    )
    f32, bf16 = mybir.dt.float32, mybir.dt.bfloat16

    # per-core inputs (same shapes everywhere, different data)
    whh = nc.dram_tensor("whh", [128, 2 * NKH * 128], bf16, kind="ExternalInput")
    wih = nc.dram_tensor("wih", [128, 2 * NKX * 128], bf16, kind="ExternalInput")
    wout = nc.dram_tensor("wout", [128, 2 * OUT_TOT], bf16, kind="ExternalInput")
    bias = nc.dram_tensor("bias", [128, 2], f32, kind="ExternalInput")
    bout = nc.dram_tensor("bout", [OUT_TOT, 1], f32, kind="ExternalInput")
    myid = nc.dram_tensor("myid", [1, 1], mybir.dt.uint32, kind="ExternalInput")
    xt = nc.dram_tensor("xt", [t_io, IN_TOT, B], bf16, kind="ExternalInput")
    out_p = nc.dram_tensor("out_p", [t_io, OUT_TOT, B], f32, kind="ExternalOutput")

    # semaphore thresholds (all static)
    def act_after(t):          # act_sem value once tanh of step t retired
        return 2 + 2 * (t + 1)  # +2 from the initial h memset

    def recv_thresh(t):        # recv[d][t&1] needed before step-t matmuls
        return 2 * ((t + 1) // 2)

    import contextlib
    with contextlib.ExitStack() as stk:
        E = stk.enter_context
        w_sem = E(nc.semaphore("w_sem"))
        x_sem = [E(nc.semaphore(f"x_sem_{s}")) for s in range(XRING)]
        out_sem = [E(nc.semaphore(f"out_sem_{par}")) for par in range(2)]
        prep_sem = E(nc.semaphore("prep_sem"))
        send_sem = [E(nc.semaphore(f"send_sem_{par}")) for par in range(2)]
        act_sem = E(nc.semaphore("act_sem"))
        pe_z_sem = E(nc.semaphore("pe_z_sem"))
        pe_r_sem = E(nc.semaphore("pe_r_sem"))
        dve_sem = E(nc.semaphore("dve_sem"))
        wh_sb = E(nc.sbuf_tensor("wh_sb", [128, 2 * NKH * 128], bf16))
        wih_sb = E(nc.sbuf_tensor("wih_sb", [128, 2 * NKX * 128], bf16))
        wout_sb = E(nc.sbuf_tensor("wout_sb", [128, 2 * OUT_TOT], bf16))
        bias_sb = E(nc.sbuf_tensor("bias_sb", [128, 2], f32))
        bout_sb = E(nc.sbuf_tensor("bout_sb", [OUT_TOT, 1], f32))
        hall0 = E(nc.sbuf_tensor("hall0", [128, H], bf16))
        hall1 = E(nc.sbuf_tensor("hall1", [128, H], bf16))
        x_sb = E(nc.sbuf_tensor("x_sb", [128, XRING * IN_TOT], bf16))
        stage = E(nc.sbuf_tensor("stage", [OUT_TOT, 2 * B], f32))
        hmine = E(nc.sbuf_tensor("hmine", [128, 2 * 256], bf16))
        id_sb = E(nc.sbuf_tensor("id_sb", [1, 1], mybir.dt.uint32))
        pz00 = E(nc.psum_tensor("pz00", [128, B], f32))
        pz10 = E(nc.psum_tensor("pz10", [128, B], f32))
        pz01 = E(nc.psum_tensor("pz01", [128, B], f32))
        pz11 = E(nc.psum_tensor("pz11", [128, B], f32))
        pr0 = E(nc.psum_tensor("pr0", [OUT_TOT, B], f32))
        pr1 = E(nc.psum_tensor("pr1", [OUT_TOT, B], f32))
        recv = [E(nc.semaphore(f"recv_{par}")) for par in range(2)]

        hall = [hall0, hall1]
        pz = [[pz00, pz01], [pz10, pz11]]   # pz[m][parity]
        pr = [pr0, pr1]

        def wh_tile(j, m):     # lhsT for K-slot j (0..15), M-chunk m
            c0 = (2 * j + m) * 128
            return wh_sb[:, c0:c0 + 128]

        def wih_tile(k, m):
            c0 = (2 * k + m) * 128
            return wih_sb[:, c0:c0 + 128]

        def wout_tile(m):
            return wout_sb[:, m * OUT_TOT:(m + 1) * OUT_TOT]

        with nc.Block() as block:

            @block.gpsimd
            def _(gp):
                gp.dma_start(out=wh_sb[:, :], in_=whh[:, :]).then_inc(w_sem, 16)
                gp.dma_start(out=wih_sb[:, :], in_=wih[:, :]).then_inc(w_sem, 16)
                gp.dma_start(out=wout_sb[:, :], in_=wout[:, :]).then_inc(w_sem, 16)
                gp.dma_start(out=bias_sb[:, :], in_=bias[:, :]).then_inc(w_sem, 16)
                gp.dma_start(out=bout_sb[:, :], in_=bout[:, :]).then_inc(w_sem, 16)
                gp.dma_start(out=id_sb[:, :], in_=myid[:, :]).then_inc(w_sem, 16)
                gp.memset(hall[0][:, :], 0).then_inc(act_sem, 2)
                gp.wait_ge(w_sem, 96)
                with gp.register("pidr") as pidr:
                    gp.reg_load(pidr, id_sb[0:1, 0:1])
                    myv = gp.snap(pidr, min_val=0, max_val=NCORES - 1)
                off = myv * 256
                rdests = [(0, d) for d in range(8)]
                for t in range(T if variant == "full" else 0):
                    par = (t + 1) & 1
                    gp.remote_dma_broadcast(
                        out_ap=hall[par][:, bass.ds(off, 256)],
                        in_ap=hmine[:, par * 256:par * 256 + 256],
                        remote_sem=recv[par],
                        local_sem=send_sem[par],
                        rdests=rdests,
                    ).then_inc(prep_sem, 1)
                    gp.wait_ge(prep_sem, t + 1)
                    gp.wait_ge(act_sem, act_after(t))
                    gp.trigger_dma(count=1)

            @block.sync
            def _(sy):
                def x_dma(tp):
                    src = xt[tp % t_io].rearrange("(k p) b -> p k b", p=128)
                    slot = tp % XRING
                    dst = x_sb[:, slot * IN_TOT:(slot + 1) * IN_TOT]
                    sy.dma_start(
                        out=dst.rearrange("p (k b) -> p k b", k=NKX),
                        in_=src,
                    ).then_inc(x_sem[slot], 16)

                for tp in range(min(XRING, T)):
                    x_dma(tp)
                for t in range(T):
                    tp = t + XRING
                    if tp < T:
                        sy.wait_ge(pe_z_sem, t + 1)
                        x_dma(tp)
                    sy.wait_ge(dve_sem, t + 1)
                    sy.dma_start(
                        out=out_p[t % t_io],
                        in_=stage[:, (t & 1) * B:(t & 1) * B + B],
                    ).then_inc(out_sem[t & 1], 16)

            @block.tensor
            def _(pe):
                pe.wait_ge(w_sem, 96)
                for t in range(T):
                    p = t & 1
                    slot = t % XRING
                    hbuf = hall[p]
                    # --- input projection (opens the accumulation groups)
                    pe.wait_ge(x_sem[slot], 16 * (t // XRING + 1))
                    if t == 2:
                        pe.wait_ge(act_sem, act_after(0))
                    for m in range(2):
                        for k in range(NKX):
                            pe.matmul(
                                pz[m][p][:, :],
                                wih_tile(k, m),
                                x_sb[:, slot * IN_TOT + 128 * k:
                                     slot * IN_TOT + 128 * (k + 1)],
                                start=(k == 0), stop=False,
                            )
                    # --- readout of h_t (tanh output of step t-1)
                    if t >= 1:
                        pe.wait_ge(act_sem, act_after(t - 1))
                        if t >= 3:
                            pe.wait_ge(dve_sem, t - 2)
                        rp = (t - 1) & 1
                        hm = hmine[:, (t & 1) * 256:(t & 1) * 256 + 256]
                        pe.matmul(pr[rp][:, :], wout_tile(0),
                                  hm[:, 0:128], start=True, stop=False)
                        pe.matmul(pr[rp][:, :], wout_tile(1),
                                  hm[:, 128:256], start=False, stop=True,
                                  ).then_inc(pe_r_sem, 1)
                    # --- all 16 h K-chunks (single arrival wait, global order)
                    th = (16 * ((t + 1) // 2)) if variant == "full" else 0
                    if th > 0:
                        pe.wait_ge(recv[p], th)
                    for j in range(NKH):
                        for m in range(2):
                            last = (j == NKH - 1)
                            ins = pe.matmul(
                                pz[m][p][:, :], wh_tile(j, m),
                                hbuf[:, 128 * j:128 * (j + 1)],
                                start=False, stop=last,
                            )
                            if last and m == 1:
                                ins.then_inc(pe_z_sem, 1)
                    # end step
                # final readout (h_T in hall[T&1] slot 0)
                pe.wait_ge(act_sem, act_after(T - 1))
                pe.wait_ge(dve_sem, T - 2)
                rp = (T - 1) & 1
                hm = hmine[:, (T & 1) * 256:(T & 1) * 256 + 256]
                pe.matmul(pr[rp][:, :], wout_tile(0), hm[:, 0:128],
                          start=True, stop=False)
                pe.matmul(pr[rp][:, :], wout_tile(1), hm[:, 128:256],
                          start=False, stop=True).then_inc(pe_r_sem, 1)

            @block.scalar
            def _(act):
                for t in range(T):
                    p = t & 1
                    act.wait_ge(pe_z_sem, t + 1)
                    if t >= 2 and variant == "full":
                        act.wait_ge(send_sem[(t + 1) & 1], 16 * (t // 2))
                    for m in range(2):
                        act.activation(
                            out=hmine[:, ((t + 1) & 1) * 256 + 128 * m:
                                      ((t + 1) & 1) * 256 + 128 * (m + 1)],
                            in_=pz[m][p][:, :],
                            func=mybir.ActivationFunctionType.Tanh,
                            bias=bias_sb[:, m:m + 1],
                        ).then_inc(act_sem, 1)

            @block.vector
            def _(ve):
                for t in range(T):
                    ve.wait_ge(pe_r_sem, t + 1)
                    if t >= 2:
                        ve.wait_ge(out_sem[t & 1], 16 * (t // 2))
                    ve.tensor_scalar_add(
                        out=stage[:, (t & 1) * B:(t & 1) * B + B],
                        in0=pr[t & 1][:, :],
                        scalar1=bout_sb[:, 0:1],
                    ).then_inc(dve_sem, 1)

    return nc


def prep_in_maps(x, w_ih, b_ih, w_hh, b_hh, w_out, b_out,
                 input_mask, hh_mask, out_mask, T=T_FULL):
    """Host-side shard/pack: fold masks, cast bf16, pre-transpose, rotate."""
    f32 = np.float32
    Wih = (np.asarray(w_ih, f32) * np.asarray(input_mask, f32))
    Whh = (np.asarray(w_hh, f32) * np.asarray(hh_mask, f32))
    Wout = (np.asarray(w_out, f32) * np.asarray(out_mask, f32))
    bsum = np.asarray(b_ih, f32) + np.asarray(b_hh, f32)
    bo = np.asarray(b_out, f32)

    x = np.asarray(x, f32)[:T]
    # [T, IN, B] bf16, contiguous
    xT = np.ascontiguousarray(x.transpose(0, 2, 1)).astype(BF16)

    in_maps = []
    for c in range(NCORES):
        r0 = HSLICE * c
        # whh tiles: K-slot j -> global hidden in-chunk g
        wh = np.empty((128, 2 * NKH, 128), BF16)
        for j in range(NKH):
            g = j
            for m in range(2):
                blk = Whh[r0 + 128 * m: r0 + 128 * (m + 1),
                          128 * g: 128 * (g + 1)]
                wh[:, 2 * j + m, :] = blk.T.astype(BF16)
        wihp = np.empty((128, 2 * NKX, 128), BF16)
        for k in range(NKX):
            for m in range(2):
                blk = Wih[r0 + 128 * m: r0 + 128 * (m + 1),
                          128 * k: 128 * (k + 1)]
                wihp[:, 2 * k + m, :] = blk.T.astype(BF16)
        wo = np.empty((128, 2, OUT_TOT), BF16)
        for m in range(2):
            wo[:, m, :] = Wout[:, r0 + 128 * m: r0 + 128 * (m + 1)].T.astype(BF16)
        bias = np.stack([bsum[r0:r0 + 128], bsum[r0 + 128:r0 + 256]],
                        axis=1).astype(f32)
        in_maps.append({
            "whh": np.ascontiguousarray(wh.reshape(128, 2 * NKH * 128)),
            "wih": np.ascontiguousarray(wihp.reshape(128, 2 * NKX * 128)),
            "wout": np.ascontiguousarray(wo.reshape(128, 2 * OUT_TOT)),
            "bias": np.ascontiguousarray(bias),
            "bout": np.ascontiguousarray((bo / NCORES).reshape(OUT_TOT, 1)),
            "myid": np.array([[c]], np.uint32),
            "xt": xT,
        })
    return in_maps


def assemble(results, T=T_FULL):
    """Sum per-core partial readouts -> [T, 4, B, 16] f32."""
    acc = np.zeros((T, OUT_TOT, B), np.float32)
    for r in results:
        acc += r["out_p"]
    # out[t, m, b, o] = acc[t, m*16+o, b]
    return np.ascontiguousarray(
        acc.reshape(T, 4, 16, B).transpose(0, 1, 3, 2))


_CACHE = {}
_LAST_RESULT = None


def kernel(**inputs) -> np.ndarray:
    global _LAST_RESULT
    import os
    T = inputs["x"].shape[0]
    if T not in _CACHE:
        nc = build(T)
        nc.finalize()
        _CACHE[T] = nc
    nc = _CACHE[T]
    in_maps = prep_in_maps(T=T, **inputs)
    trace = bool(int(os.environ.get("BASS_RNN_TRACE", "0")))
    res = run_bass_kernel_spmd(
        nc, in_maps, core_ids=list(range(NCORES)), trace=trace
    )
    _LAST_RESULT = res
    return assemble(res.results, T=T)
